# revision 15
# baseline (speedup 1.0000x reference)
"""Distance-transform kernel for Trainium2 (Bass/Tile), 8-core data parallel.

Reference semantics (per (B*C) image, 128x128):
  repeat n times:  s = conv3x3_replicate(boundary, K);  cdt = -h*log(s) (posinf->0)
                   out += (cdt>0) ? i + cdt : 0;  boundary |= (cdt>0)
with K[dy,dx] = exp(-hypot(dx,dy)/h). boundary is binary, so each pixel is
updated exactly once (at first touch), and once boundary saturates all later
iterations contribute zero. Reformulated:
  S  = conv value at first touch   (predicated copy while pixel untouched)
  T' = -sum of boundary masks      (first-touch index = n + T')
  out = (S>0) ? (n + T') - h*log(S) : 0
The 3x3 kernel is rank-2: conv(B) = M1 @ B + M2 @ (shiftL(B)+shiftR(B)) where
M1/M2 are tridiagonal 128x128 (replicate boundary folded in) -> two PE matmuls
per iteration; the horizontal replicate is folded into a split shift-add
(interior + edge columns). The trip count is data-dependent (boundary
saturation); it is computed on the host from the input with a capped dilation
loop (cap 128 = reference trip count), which is exact by the early-exit
argument above.

Sharding: 24 images split 3-per-core across 8 cores, no communication.
"""

import math

import numpy as np

H_PARAM = 0.35
_PROGRAM_CACHE = {}


def _make_mats():
    a = 1.0
    b = math.exp(-1.0 / H_PARAM)
    c = math.exp(-math.sqrt(2.0) / H_PARAM)
    M1 = np.zeros((128, 128), dtype=np.float64)
    M2 = np.zeros((128, 128), dtype=np.float64)
    i = np.arange(128)
    M1[i, i] = a
    M2[i, i] = b
    M1[i[1:], i[1:] - 1] = b
    M1[i[:-1], i[:-1] + 1] = b
    M2[i[1:], i[1:] - 1] = c
    M2[i[:-1], i[:-1] + 1] = c
    M1[0, 0] += b
    M1[127, 127] += b
    M2[0, 0] += c
    M2[127, 127] += c
    return M1.astype(np.float16), M2.astype(np.float16)


def _needed_iters(flat):
    """Dilation steps until the binary mask saturates; capped at the
    reference trip count (128). flat: (N,128,128) float."""
    B = flat > 0
    n = 0
    while n < 128 and not B.all():
        P = np.pad(B, ((0, 0), (1, 1), (1, 1)), mode="edge")
        D = np.zeros_like(B)
        for dy in range(3):
            for dx in range(3):
                D |= P[:, dy : dy + 128, dx : dx + 128]
        B = D
        n += 1
    return n


def _build(n_iters):
    import concourse.bacc as bacc
    import concourse.tile as tile
    from concourse import mybir
    from concourse.alu_op_type import AluOpType as alu

    f32 = mybir.dt.float32
    f16 = mybir.dt.float16
    u16 = mybir.dt.uint16

    nc = bacc.Bacc(
        "TRN2",
        target_bir_lowering=False,
        debug=False,
        enable_asserts=False,
        num_devices=8,
    )
    img = nc.dram_tensor("image", [3, 128, 128], f32, kind="ExternalInput")
    m1d = nc.dram_tensor("m1", [128, 128], f16, kind="ExternalInput")
    m2d = nc.dram_tensor("m2", [128, 128], f16, kind="ExternalInput")
    outd = nc.dram_tensor("out", [3, 128, 128], f32, kind="ExternalOutput")

    PW = 130  # per-image width; cols 1..128 active, cols 0/129 junk
    FW = 3 * PW

    def v3(t):  # [128, FW] tile -> [128, 3, PW] view
        return t[:].rearrange("p (c w) -> p c w", c=3)

    with tile.TileContext(nc) as tc:
        with (
            tc.tile_pool(name="state", bufs=1) as st,
            tc.tile_pool(name="work", bufs=3) as wk,
            tc.tile_pool(name="psum", bufs=4, space="PSUM") as pp,
        ):
            M1s = st.tile([128, 128], f16, name="M1s")
            M2s = st.tile([128, 128], f16, name="M2s")
            nc.sync.dma_start(M1s[:], m1d.ap())
            nc.sync.dma_start(M2s[:], m2d.ap())

            Bt = [st.tile([128, FW], f16, name=f"B{k}") for k in range(3)]
            Gt = [st.tile([128, FW], f16, name=f"G{k}") for k in range(3)]
            St = [st.tile([128, FW], f32, name=f"S{k}") for k in range(3)]
            T = st.tile([128, FW], f16, name="T")

            x_sb = wk.tile([128, 3 * 128], f32, tag="x")
            nc.sync.dma_start(
                x_sb[:].rearrange("p (c w) -> p c w", c=3),
                img.ap().rearrange("c h w -> h c w"),
            )
            for k in range(3):
                nc.vector.memset(Bt[k][:], 0.0)
                nc.vector.memset(Gt[k][:], 0.0)
            nc.vector.tensor_copy(
                v3(Bt[0])[:, :, 1:129], x_sb[:].rearrange("p (c w) -> p c w", c=3)
            )
            nc.vector.memset(St[0][:], 0.0)
            nc.vector.memset(T[:], 0.0)

            # Software-pipelined emission: the S/T updates of iteration i-1
            # are emitted during iteration i so the in-order engine queues
            # keep the critical cycle (is_gt -> G -> mm2 -> is_gt) tight.
            def emit_updates(j):
                # S_new = where(B_old, S_old, s): ACT staged s into S[j+1];
                # restore the already-touched entries. Mask must be int-typed
                # for the BIR verifier; fp16 {0,1} bitcast to u16.
                nc.vector.copy_predicated(
                    v3(St[(j + 1) % 3])[:, :, 1:129],
                    v3(Bt[j % 3]).bitcast(u16)[:, :, 1:129],
                    v3(St[j % 3])[:, :, 1:129],
                )
                # T' -= B_new  (T = n + T' applied in the epilogue)
                nc.gpsimd.tensor_tensor(
                    v3(T)[:, :, 1:129],
                    v3(T)[:, :, 1:129],
                    v3(Bt[(j + 1) % 3])[:, :, 1:129],
                    op=alu.subtract,
                )

            for i in range(n_iters):
                Bp, Bn = Bt[i % 3], Bt[(i + 1) % 3]
                Bv, Bnv = v3(Bp), v3(Bn)
                G = Gt[i % 3]
                Gv = v3(G)
                ps = pp.tile([128, FW], f32, tag="ps")
                psv = v3(ps)
                if i > 0:
                    # B_new = dilate8(B) = (s > 0); emitted here (not after
                    # the matmuls) so DVE's in-order queue runs it first
                    nc.vector.tensor_scalar(
                        Bv[:, :, 1:129],
                        v3(pprev)[:, :, 1:129],
                        0.0,
                        None,
                        op0=alu.is_gt,
                    )
                # G interior: G[w] = B[w-1] + B[w+1], w in 2..127
                nc.vector.tensor_tensor(
                    Gv[:, :, 2:128], Bv[:, :, 1:127], Bv[:, :, 3:129], op=alu.add
                )
                # G edges (horizontal replicate): G[1] = B[1]+B[2],
                # G[128] = B[127]+B[128]; cols {1,128} per image in one op
                nc.vector.tensor_tensor(
                    Gv[:, :, 1:129:127],
                    Bv[:, :, 1:128:126],
                    Bv[:, :, 2:129:126],
                    op=alu.add,
                )
                # conv: full-width matmuls (pad-column results are junk and
                # never consumed; G slots are pre-initialized)
                nc.tensor.matmul(ps[:], M1s[:], Bp[:], start=True, stop=False)
                nc.tensor.matmul(ps[:], M2s[:], G[:], start=False, stop=True)
                # stage s into the next S tile (fixed up by emit_updates)
                nc.scalar.activation(
                    v3(St[(i + 1) % 3])[:, :, 1:129],
                    psv[:, :, 1:129],
                    mybir.ActivationFunctionType.Copy,
                )
                if i > 0:
                    emit_updates(i - 1)
                pprev = ps
            # final B update + last S/T fixups
            nc.vector.tensor_scalar(
                v3(Bt[n_iters % 3])[:, :, 1:129],
                v3(pprev)[:, :, 1:129],
                0.0,
                None,
                op0=alu.is_gt,
            )
            emit_updates(n_iters - 1)

            S = St[n_iters % 3]
            act = lambda t: v3(t)[:, :, 1:129]
            Sc = wk.tile([128, FW], f32, tag="fin_a")
            nc.vector.tensor_scalar_max(act(Sc), act(S), 1e-30)
            lnS = wk.tile([128, FW], f32, tag="fin_b")
            nc.scalar.activation(
                act(lnS), act(Sc), mybir.ActivationFunctionType.Ln
            )
            tmp = wk.tile([128, FW], f32, tag="fin_c")
            nc.vector.scalar_tensor_tensor(
                act(tmp), act(lnS), -H_PARAM, act(T), op0=alu.mult, op1=alu.add
            )
            # true T = n_iters + T'; fold the offset here
            tmp2 = wk.tile([128, FW], f32, tag="fin_d")
            nc.vector.tensor_scalar_add(act(tmp2), act(tmp), float(n_iters))
            outv = wk.tile([128, FW], f32, tag="fin_e")
            nc.vector.scalar_tensor_tensor(
                act(outv), act(S), 0.0, act(tmp2), op0=alu.is_gt, op1=alu.mult
            )
            nc.sync.dma_start(
                outd.ap().rearrange("c h w -> h c w"), act(outv)
            )

    nc.compile()
    return nc


def _get_program(n_iters):
    if n_iters not in _PROGRAM_CACHE:
        _PROGRAM_CACHE[n_iters] = _build(n_iters)
    return _PROGRAM_CACHE[n_iters]


def kernel(image):
    from concourse.bass_utils import run_bass_kernel_spmd

    image = np.ascontiguousarray(np.asarray(image), dtype=np.float32)
    assert image.shape == (8, 3, 128, 128)
    n = _needed_iters(image.reshape(24, 128, 128))
    nc = _get_program(n)
    M1np, M2np = _make_mats()
    in_maps = [
        {"image": image[c], "m1": M1np, "m2": M2np} for c in range(8)
    ]
    res = run_bass_kernel_spmd(nc, in_maps, core_ids=list(range(8)))
    return np.stack([res.results[c]["out"] for c in range(8)]).astype(
        np.float32
    )


# revision 19
# speedup vs baseline: 1.1118x; 1.1118x over previous
"""Distance-transform kernel for Trainium2 (Bass/Tile), 8-core data parallel.

Reference semantics (per (B*C) image, 128x128):
  repeat n times:  s = conv3x3_replicate(boundary, K);  cdt = -h*log(s) (posinf->0)
                   out += (cdt>0) ? i + cdt : 0;  boundary |= (cdt>0)
with K[dy,dx] = exp(-hypot(dx,dy)/h). boundary is binary, so each pixel is
updated exactly once (at first touch), and once boundary saturates all later
iterations contribute zero. Reformulated:
  S  = conv value at first touch   (predicated copy while pixel untouched)
  T' = -sum of boundary masks      (first-touch index = n + T')
  out = (S>0) ? (n + T') - h*log(S) : 0
The 3x3 kernel is rank-2: conv(B) = M1 @ B + M2 @ (shiftL(B)+shiftR(B)) where
M1/M2 are tridiagonal 128x128 (replicate boundary folded in) -> two PE matmuls
per iteration; the horizontal replicate is folded into a split shift-add
(interior + edge columns). The trip count is data-dependent (boundary
saturation); it is computed on the host from the input with a capped dilation
loop (cap 128 = reference trip count), which is exact by the early-exit
argument above.

Sharding: 24 images split 3-per-core across 8 cores, no communication.
"""

import math

import numpy as np

H_PARAM = 0.35
_PROGRAM_CACHE = {}


def _make_mats():
    a = 1.0
    b = math.exp(-1.0 / H_PARAM)
    c = math.exp(-math.sqrt(2.0) / H_PARAM)
    M1 = np.zeros((128, 128), dtype=np.float64)
    M2 = np.zeros((128, 128), dtype=np.float64)
    i = np.arange(128)
    M1[i, i] = a
    M2[i, i] = b
    M1[i[1:], i[1:] - 1] = b
    M1[i[:-1], i[:-1] + 1] = b
    M2[i[1:], i[1:] - 1] = c
    M2[i[:-1], i[:-1] + 1] = c
    M1[0, 0] += b
    M1[127, 127] += b
    M2[0, 0] += c
    M2[127, 127] += c
    return M1.astype(np.float16), M2.astype(np.float16)


def _needed_iters(flat):
    """Dilation steps until the binary mask saturates; capped at the
    reference trip count (128). flat: (N,128,128) float."""
    B = flat > 0
    n = 0
    while n < 128 and not B.all():
        P = np.pad(B, ((0, 0), (1, 1), (1, 1)), mode="edge")
        D = np.zeros_like(B)
        for dy in range(3):
            for dx in range(3):
                D |= P[:, dy : dy + 128, dx : dx + 128]
        B = D
        n += 1
    return n


def _build(n_iters):
    import concourse.bacc as bacc
    import concourse.tile as tile
    from concourse import mybir
    from concourse.alu_op_type import AluOpType as alu

    f32 = mybir.dt.float32
    f16 = mybir.dt.float16
    u16 = mybir.dt.uint16

    nc = bacc.Bacc(
        "TRN2",
        target_bir_lowering=False,
        debug=False,
        enable_asserts=False,
        num_devices=8,
    )
    img = nc.dram_tensor("image", [3, 128, 128], f32, kind="ExternalInput")
    m1d = nc.dram_tensor("m1", [128, 128], f16, kind="ExternalInput")
    m2d = nc.dram_tensor("m2", [128, 128], f16, kind="ExternalInput")
    outd = nc.dram_tensor("out", [3, 128, 128], f32, kind="ExternalOutput")

    PW = 130  # per-image width; cols 1..128 active, cols 0/129 junk
    FW = 3 * PW

    def v3(t):  # [128, FW] tile -> [128, 3, PW] view
        return t[:].rearrange("p (c w) -> p c w", c=3)

    with tile.TileContext(nc) as tc:
        with (
            tc.tile_pool(name="state", bufs=1) as st,
            tc.tile_pool(name="work", bufs=3) as wk,
            tc.tile_pool(name="psum", bufs=4, space="PSUM") as pp,
        ):
            M1s = st.tile([128, 128], f16, name="M1s")
            M2s = st.tile([128, 128], f16, name="M2s")
            nc.sync.dma_start(M1s[:], m1d.ap())
            nc.sync.dma_start(M2s[:], m2d.ap())

            Bt = [st.tile([128, FW], f16, name=f"B{k}") for k in range(3)]
            Gt = [st.tile([128, FW], f16, name=f"G{k}") for k in range(3)]
            St = [st.tile([128, FW], f32, name=f"S{k}") for k in range(3)]
            T = st.tile([128, FW], f16, name="T")

            x_sb = wk.tile([128, 3 * 128], f32, tag="x")
            nc.sync.dma_start(
                x_sb[:].rearrange("p (c w) -> p c w", c=3),
                img.ap().rearrange("c h w -> h c w"),
            )
            for k in range(3):
                nc.vector.memset(Bt[k][:], 0.0)
                nc.vector.memset(Gt[k][:], 0.0)
            nc.vector.tensor_copy(
                v3(Bt[0])[:, :, 1:129], x_sb[:].rearrange("p (c w) -> p c w", c=3)
            )
            nc.vector.memset(St[0][:], 0.0)
            nc.vector.memset(T[:], 0.0)

            # Software-pipelined emission: the S/T updates of iteration i-1
            # are emitted during iteration i so the in-order engine queues
            # keep the critical cycle (is_gt -> G -> mm2 -> is_gt) tight.
            from concourse.tile import add_dep_helper

            def emit_updates(j, after=None):
                # S_new = where(B_old, S_old, s): ACT staged s into S[j+1];
                # restore the already-touched entries. Mask must be int-typed
                # for the BIR verifier; fp16 {0,1} bitcast to u16.
                cp = nc.vector.copy_predicated(
                    v3(St[(j + 1) % 3])[:, :, 1:129],
                    v3(Bt[j % 3]).bitcast(u16)[:, :, 1:129],
                    v3(St[j % 3])[:, :, 1:129],
                )
                if after is not None:
                    # order-only edge: keep the S fixup behind the next
                    # iteration's critical DVE ops in the in-order queue
                    add_dep_helper(
                        cp.ins, after.ins, sync=False, reason="cpred after next G"
                    )
                # T' -= B_new  (T = n + T' applied in the epilogue)
                nc.gpsimd.tensor_tensor(
                    v3(T)[:, :, 1:129],
                    v3(T)[:, :, 1:129],
                    v3(Bt[(j + 1) % 3])[:, :, 1:129],
                    op=alu.subtract,
                )

            for i in range(n_iters):
                Bp, Bn = Bt[i % 3], Bt[(i + 1) % 3]
                Bv, Bnv = v3(Bp), v3(Bn)
                G = Gt[i % 3]
                Gv = v3(G)
                ps = pp.tile([128, FW], f32, tag="ps")
                psv = v3(ps)
                if i > 0:
                    # B_new = dilate8(B) = (s > 0); emitted here (not after
                    # the matmuls) so DVE's in-order queue runs it first
                    nc.vector.tensor_scalar(
                        Bv[:, :, 1:129],
                        v3(pprev)[:, :, 1:129],
                        0.0,
                        None,
                        op0=alu.is_gt,
                    )
                # G main: G[w] = B[w-1] + B[w+1] for w in 1..128 at full
                # 128-wide inner count (DVE fast mode); edge cols {1,128}
                # get junk from the zero pad cols and are overwritten below
                nc.vector.tensor_tensor(
                    Gv[:, :, 1:129], Bv[:, :, 0:128], Bv[:, :, 2:130], op=alu.add
                )
                # G edges (horizontal replicate): G[1] = B[1]+B[2],
                # G[128] = B[127]+B[128]; cols {1,128} per image in one op
                g_edge = nc.vector.tensor_tensor(
                    Gv[:, :, 1:129:127],
                    Bv[:, :, 1:128:126],
                    Bv[:, :, 2:129:126],
                    op=alu.add,
                )
                # conv: full-width matmuls (pad-column results are junk and
                # never consumed; G slots are pre-initialized)
                nc.tensor.matmul(ps[:], M1s[:], Bp[:], start=True, stop=False)
                nc.tensor.matmul(ps[:], M2s[:], G[:], start=False, stop=True)
                # stage s into the next S tile (fixed up by emit_updates)
                nc.scalar.activation(
                    v3(St[(i + 1) % 3])[:, :, 1:129],
                    psv[:, :, 1:129],
                    mybir.ActivationFunctionType.Copy,
                )
                if i > 0:
                    emit_updates(i - 1, after=g_edge)
                pprev = ps
            # final B update + last S/T fixups
            nc.vector.tensor_scalar(
                v3(Bt[n_iters % 3])[:, :, 1:129],
                v3(pprev)[:, :, 1:129],
                0.0,
                None,
                op0=alu.is_gt,
            )
            emit_updates(n_iters - 1)

            S = St[n_iters % 3]
            act = lambda t: v3(t)[:, :, 1:129]
            Sc = wk.tile([128, FW], f32, tag="fin_a")
            nc.vector.tensor_scalar_max(act(Sc), act(S), 1e-30)
            lnS = wk.tile([128, FW], f32, tag="fin_b")
            nc.scalar.activation(
                act(lnS), act(Sc), mybir.ActivationFunctionType.Ln
            )
            tmp = wk.tile([128, FW], f32, tag="fin_c")
            nc.vector.scalar_tensor_tensor(
                act(tmp), act(lnS), -H_PARAM, act(T), op0=alu.mult, op1=alu.add
            )
            # true T = n_iters + T'; fold the offset here
            tmp2 = wk.tile([128, FW], f32, tag="fin_d")
            nc.vector.tensor_scalar_add(act(tmp2), act(tmp), float(n_iters))
            outv = wk.tile([128, FW], f32, tag="fin_e")
            nc.vector.scalar_tensor_tensor(
                act(outv), act(S), 0.0, act(tmp2), op0=alu.is_gt, op1=alu.mult
            )
            nc.sync.dma_start(
                outd.ap().rearrange("c h w -> h c w"), act(outv)
            )

    nc.compile()
    return nc


def _get_program(n_iters):
    if n_iters not in _PROGRAM_CACHE:
        _PROGRAM_CACHE[n_iters] = _build(n_iters)
    return _PROGRAM_CACHE[n_iters]


def kernel(image):
    from concourse.bass_utils import run_bass_kernel_spmd

    image = np.ascontiguousarray(np.asarray(image), dtype=np.float32)
    assert image.shape == (8, 3, 128, 128)
    n = _needed_iters(image.reshape(24, 128, 128))
    nc = _get_program(n)
    M1np, M2np = _make_mats()
    in_maps = [
        {"image": image[c], "m1": M1np, "m2": M2np} for c in range(8)
    ]
    res = run_bass_kernel_spmd(nc, in_maps, core_ids=list(range(8)))
    return np.stack([res.results[c]["out"] for c in range(8)]).astype(
        np.float32
    )


# revision 23
# speedup vs baseline: 1.2647x; 1.1374x over previous
"""Distance-transform kernel for Trainium2 (Bass/Tile), 8-core data parallel.

Reference semantics (per (B*C) image, 128x128):
  repeat n times:  s = conv3x3_replicate(boundary, K);  cdt = -h*log(s) (posinf->0)
                   out += (cdt>0) ? i + cdt : 0;  boundary |= (cdt>0)
with K[dy,dx] = exp(-hypot(dx,dy)/h). boundary is binary, so each pixel is
updated exactly once (at first touch), and once boundary saturates all later
iterations contribute zero. Reformulated:
  S  = conv value at first touch   (predicated copy while pixel untouched)
  T' = -sum of boundary masks      (first-touch index = n + T')
  out = (S>0) ? (n + T') - h*log(S) : 0
The 3x3 kernel is rank-2: conv(B) = M1 @ B + M2 @ (shiftL(B)+shiftR(B)) where
M1/M2 are tridiagonal 128x128 (replicate boundary folded in) -> two PE matmuls
per iteration; the horizontal replicate is folded into a split shift-add
(interior + edge columns). The trip count is data-dependent (boundary
saturation); it is computed on the host from the input with a capped dilation
loop (cap 128 = reference trip count), which is exact by the early-exit
argument above.

Sharding: 24 images split 3-per-core across 8 cores, no communication.
"""

import math

import numpy as np

H_PARAM = 0.35
_PROGRAM_CACHE = {}


def _make_mats():
    a = 1.0
    b = math.exp(-1.0 / H_PARAM)
    c = math.exp(-math.sqrt(2.0) / H_PARAM)
    M1 = np.zeros((128, 128), dtype=np.float64)
    M2 = np.zeros((128, 128), dtype=np.float64)
    i = np.arange(128)
    M1[i, i] = a
    M2[i, i] = b
    M1[i[1:], i[1:] - 1] = b
    M1[i[:-1], i[:-1] + 1] = b
    M2[i[1:], i[1:] - 1] = c
    M2[i[:-1], i[:-1] + 1] = c
    M1[0, 0] += b
    M1[127, 127] += b
    M2[0, 0] += c
    M2[127, 127] += c
    return M1.astype(np.float16), M2.astype(np.float16)


def _needed_iters(flat):
    """Dilation steps until the binary mask saturates; capped at the
    reference trip count (128). flat: (N,128,128) float."""
    B = flat > 0
    n = 0
    while n < 128 and not B.all():
        P = np.pad(B, ((0, 0), (1, 1), (1, 1)), mode="edge")
        D = np.zeros_like(B)
        for dy in range(3):
            for dx in range(3):
                D |= P[:, dy : dy + 128, dx : dx + 128]
        B = D
        n += 1
    return n


def _build(n_iters):
    import concourse.bacc as bacc
    import concourse.tile as tile
    from concourse import mybir
    from concourse.alu_op_type import AluOpType as alu

    f32 = mybir.dt.float32
    f16 = mybir.dt.float16
    u16 = mybir.dt.uint16

    nc = bacc.Bacc(
        "TRN2",
        target_bir_lowering=False,
        debug=False,
        enable_asserts=False,
        num_devices=8,
    )
    img = nc.dram_tensor("image", [3, 128, 128], f32, kind="ExternalInput")
    m1d = nc.dram_tensor("m1", [128, 128], f16, kind="ExternalInput")
    m2d = nc.dram_tensor("m2", [128, 128], f16, kind="ExternalInput")
    outd = nc.dram_tensor("out", [3, 128, 128], f32, kind="ExternalOutput")

    PW = 130  # per-image width; cols 1..128 active, cols 0/129 junk
    FW = 3 * PW

    def v3(t):  # [128, FW] tile -> [128, 3, PW] view
        return t[:].rearrange("p (c w) -> p c w", c=3)

    with tile.TileContext(nc) as tc:
        with (
            tc.tile_pool(name="state", bufs=1) as st,
            tc.tile_pool(name="work", bufs=3) as wk,
            tc.tile_pool(name="psum", bufs=4, space="PSUM") as pp,
        ):
            M1s = st.tile([128, 128], f16, name="M1s")
            M2s = st.tile([128, 128], f16, name="M2s")
            nc.sync.dma_start(M1s[:], m1d.ap())
            nc.sync.dma_start(M2s[:], m2d.ap())

            Bt = [st.tile([128, FW], f16, name=f"B{k}") for k in range(3)]
            Gt = [st.tile([128, FW], f16, name=f"G{k}") for k in range(3)]
            St = [st.tile([128, FW], f32, name=f"S{k}") for k in range(3)]
            T = st.tile([128, FW], f16, name="T")

            x_sb = wk.tile([128, 3 * 128], f32, tag="x")
            nc.sync.dma_start(
                x_sb[:].rearrange("p (c w) -> p c w", c=3),
                img.ap().rearrange("c h w -> h c w"),
            )
            for k in range(3):
                nc.vector.memset(Bt[k][:], 0.0)
                nc.vector.memset(Gt[k][:], 0.0)
            nc.vector.tensor_copy(
                v3(Bt[0])[:, :, 1:129], x_sb[:].rearrange("p (c w) -> p c w", c=3)
            )
            nc.vector.tensor_copy(
                v3(Bt[0])[:, :, 0:130:129], v3(Bt[0])[:, :, 1:129:127]
            )
            nc.vector.memset(St[0][:], 0.0)
            nc.vector.memset(T[:], 0.0)

            # Software-pipelined emission: the S/T updates of iteration i-1
            # are emitted during iteration i so the in-order engine queues
            # keep the critical cycle (is_gt -> G -> mm2 -> is_gt) tight.
            from concourse.tile import add_dep_helper

            def emit_updates(j, after=None):
                # S_new = where(B_old, S_old, s): ACT staged s into S[j+1];
                # restore the already-touched entries. Mask must be int-typed
                # for the BIR verifier; fp16 {0,1} bitcast to u16.
                cp = nc.vector.copy_predicated(
                    v3(St[(j + 1) % 3])[:, :, 1:129],
                    v3(Bt[j % 3]).bitcast(u16)[:, :, 1:129],
                    v3(St[j % 3])[:, :, 1:129],
                )
                if after is not None:
                    # order-only edge: keep the S fixup behind the next
                    # iteration's critical DVE ops in the in-order queue
                    add_dep_helper(
                        cp.ins, after.ins, sync=False, reason="cpred after next G"
                    )
                # T' -= B_new  (T = n + T' applied in the epilogue)
                nc.gpsimd.tensor_tensor(
                    v3(T)[:, :, 1:129],
                    v3(T)[:, :, 1:129],
                    v3(Bt[(j + 1) % 3])[:, :, 1:129],
                    op=alu.subtract,
                )

            for i in range(n_iters):
                Bp, Bn = Bt[i % 3], Bt[(i + 1) % 3]
                Bv, Bnv = v3(Bp), v3(Bn)
                G = Gt[i % 3]
                Gv = v3(G)
                ps = pp.tile([128, FW], f32, tag="ps")
                psv = v3(ps)
                if i > 0:
                    # B_new = dilate8(B) = (s > 0); emitted here (not after
                    # the matmuls) so DVE's in-order queue runs it first
                    nc.vector.tensor_scalar(
                        Bv[:, :, 1:129],
                        v3(pprev)[:, :, 1:129],
                        0.0,
                        None,
                        op0=alu.is_gt,
                    )
                    # replicate pads on B via ACT (parallel with is_gt):
                    # B[0]=sign(s[1])=B[1], B[129]=sign(s[128])=B[128]
                    nc.scalar.sign(
                        Bv[:, :, 0:130:129], v3(pprev)[:, :, 1:129:127]
                    )
                # G main: G[w] = B[w-1] + B[w+1] for w in 1..128 at full
                # 128-wide inner count (DVE fast mode); exact because B's
                # pad cols 0/129 hold true replicate copies
                g_main = nc.vector.tensor_tensor(
                    Gv[:, :, 1:129], Bv[:, :, 0:128], Bv[:, :, 2:130], op=alu.add
                )
                # conv: full-width matmuls (pad-column results are junk and
                # never consumed; G slots are pre-initialized)
                nc.tensor.matmul(ps[:], M1s[:], Bp[:], start=True, stop=False)
                nc.tensor.matmul(ps[:], M2s[:], G[:], start=False, stop=True)
                # stage s into the next S tile (fixed up by emit_updates)
                nc.scalar.activation(
                    v3(St[(i + 1) % 3])[:, :, 1:129],
                    psv[:, :, 1:129],
                    mybir.ActivationFunctionType.Copy,
                )
                if i > 0:
                    emit_updates(i - 1, after=g_main)
                pprev = ps
            # final B update + last S/T fixups
            nc.vector.tensor_scalar(
                v3(Bt[n_iters % 3])[:, :, 1:129],
                v3(pprev)[:, :, 1:129],
                0.0,
                None,
                op0=alu.is_gt,
            )
            emit_updates(n_iters - 1)

            S = St[n_iters % 3]
            act = lambda t: v3(t)[:, :, 1:129]
            Sc = wk.tile([128, FW], f32, tag="fin_a")
            nc.vector.tensor_scalar_max(act(Sc), act(S), 1e-30)
            lnS = wk.tile([128, FW], f32, tag="fin_b")
            nc.scalar.activation(
                act(lnS), act(Sc), mybir.ActivationFunctionType.Ln
            )
            tmp = wk.tile([128, FW], f32, tag="fin_c")
            nc.vector.scalar_tensor_tensor(
                act(tmp), act(lnS), -H_PARAM, act(T), op0=alu.mult, op1=alu.add
            )
            # true T = n_iters + T'; fold the offset here
            tmp2 = wk.tile([128, FW], f32, tag="fin_d")
            nc.vector.tensor_scalar_add(act(tmp2), act(tmp), float(n_iters))
            outv = wk.tile([128, FW], f32, tag="fin_e")
            nc.vector.scalar_tensor_tensor(
                act(outv), act(S), 0.0, act(tmp2), op0=alu.is_gt, op1=alu.mult
            )
            nc.sync.dma_start(
                outd.ap().rearrange("c h w -> h c w"), act(outv)
            )

    nc.compile()
    return nc


def _get_program(n_iters):
    if n_iters not in _PROGRAM_CACHE:
        _PROGRAM_CACHE[n_iters] = _build(n_iters)
    return _PROGRAM_CACHE[n_iters]


def kernel(image):
    from concourse.bass_utils import run_bass_kernel_spmd

    image = np.ascontiguousarray(np.asarray(image), dtype=np.float32)
    assert image.shape == (8, 3, 128, 128)
    n = _needed_iters(image.reshape(24, 128, 128))
    nc = _get_program(n)
    M1np, M2np = _make_mats()
    in_maps = [
        {"image": image[c], "m1": M1np, "m2": M2np} for c in range(8)
    ]
    res = run_bass_kernel_spmd(nc, in_maps, core_ids=list(range(8)))
    return np.stack([res.results[c]["out"] for c in range(8)]).astype(
        np.float32
    )
